# revision 1
# baseline (speedup 1.0000x reference)
"""Trainium2 Bass kernel for nn_Block_48610439856264 (DiT-style transformer block).

B=4, N=2048, C=512, H=8 heads, D=64, d_ff=2048, fp32 I/O.

Sharding: 8 cores = 4 batches x 2 token-halves. Each core receives the full
batch (own token half first) so k/v/s are computed locally over all 2048
tokens (duplicated across the 2 cores of a batch; no collectives), while
q/gate/proj/MLP/output cover only the core's own 1024 tokens.

Device pipeline per core:
  LN1(x), LN2(e) token-major (bn_stats) -> bf16 -> xbar transpose to c-major
  kk^T = w_k^T zx^T + w_s^T ze^T (PSUM-accumulated), q^T, gate^T (c-major)
  v token-major (lhsT = zx^T blocks), augmented with a ones column per head
  scores S^T[nk,nq] = (kk^T tile).T @ q^T, exp on ACT -> E^T bf16
  attn@v: lhsT=[v_h | 1] -> [U^T; den] in PSUM, accumulate over nk tiles
  o_g^T = U^T * (1/den broadcast) * gate^T
  proj token-major (lhsT = o_g^T blocks) + residual -> x_mid
  LN3 -> z3^T -> fc1 (c-major) -> Gelu -> h^T -> fc2 token-major + residual
"""

import numpy as np
import ml_dtypes

N_CORES = 8
B, N, C = 4, 2048, 512
H, D = 8, 64
DFF = 4 * C
P = 128
NT = N // P          # 16 full-token tiles
NTO = NT // 2        # 8 own-token tiles
CT = C // P          # 4 channel tiles
DFT = DFF // P       # 16 d_ff tiles
TOK_OWN = N // 2     # 1024
EPS = 1e-5
NCH_FULL = N // 512      # 4 chunks of 512 tokens
NCH_OWN = TOK_OWN // 512  # 2

_CACHE = {}


def _build_nc():
    import concourse.bacc as bacc
    import concourse.mybir as mybir
    import concourse.tile as tile

    FP32 = mybir.dt.float32
    BF16 = mybir.dt.bfloat16

    nc = bacc.Bacc("TRN2", num_devices=N_CORES)

    # ---- DRAM I/O ----
    xb_d = nc.dram_tensor("xb", [N, C], FP32, kind="ExternalInput").ap()
    eb_d = nc.dram_tensor("eb", [N, C], FP32, kind="ExternalInput").ap()
    wqkv_d = nc.dram_tensor("wqkv", [C, 3 * C], BF16, kind="ExternalInput").ap()
    ws_d = nc.dram_tensor("ws", [C, C], BF16, kind="ExternalInput").ap()
    wgate_d = nc.dram_tensor("wgate", [C, C], BF16, kind="ExternalInput").ap()
    wproj_d = nc.dram_tensor("wproj", [C, C], BF16, kind="ExternalInput").ap()
    wfc1_d = nc.dram_tensor("wfc1", [C, DFF], BF16, kind="ExternalInput").ap()
    wfc2_d = nc.dram_tensor("wfc2", [DFF, C], BF16, kind="ExternalInput").ap()
    out_d = nc.dram_tensor("out", [TOK_OWN, C], FP32, kind="ExternalOutput").ap()

    with tile.TileContext(nc) as tc:
        _build_body(nc, tc, mybir,
                    xb_d, eb_d, out_d,
                    wqkv_d, ws_d, wgate_d, wproj_d, wfc1_d, wfc2_d)

    nc.compile()
    return nc


def _build_body(nc, tc, mybir,
                xb_r_, eb_r_, out_d,
                wqkv_d, ws_d, wgate_d, wproj_d, wfc1_d, wfc2_d):
    from contextlib import ExitStack
    from concourse.masks import make_identity

    FP32 = mybir.dt.float32
    BF16 = mybir.dt.bfloat16
    Act = mybir.ActivationFunctionType
    Alu = mybir.AluOpType

    xb_r = xb_r_.rearrange("(t p) c -> t p c", p=P)
    eb_r = eb_r_.rearrange("(t p) c -> t p c", p=P)
    out_r = out_d.rearrange("(t p) c -> t p c", p=P)

    L0 = ExitStack()
    with L0:
        consts = L0.enter_context(tc.tile_pool(name="consts", bufs=1))
        stat_pool = L0.enter_context(tc.tile_pool(name="stats", bufs=6))
        z_pool = L0.enter_context(tc.tile_pool(name="zp", bufs=4))
        xo_pool = L0.enter_context(tc.tile_pool(name="xo", bufs=NTO))
        # long-lived attention inputs/outputs
        kkT = [L0.enter_context(tc.tile_pool(name=f"kkT{i}", bufs=1))
               .tile([P, N], BF16, name=f"kkTt{i}") for i in range(CT)]
        qT = [L0.enter_context(tc.tile_pool(name=f"qT{i}", bufs=1))
              .tile([P, TOK_OWN], BF16, name=f"qTt{i}") for i in range(CT)]
        gTh = L0.enter_context(tc.tile_pool(name="gTh", bufs=1)) \
            .tile([D, H, TOK_OWN], BF16, name="gTht")
        vpool = L0.enter_context(tc.tile_pool(name="vp", bufs=NT))
        VPAD = H * (D + 1) + P  # pad so lhsT can always span 128 columns
        v_aug = [vpool.tile([P, VPAD], BF16, name=f"vaug{i}", tag="vaug")
                 for i in range(NT)]
        ogT = [L0.enter_context(tc.tile_pool(name=f"ogT{i}", bufs=1))
               .tile([P, TOK_OWN], BF16, name=f"ogTt{i}") for i in range(CT)]
        dram_p = L0.enter_context(tc.tile_pool(name="zdram", bufs=1, space="DRAM"))
        # scores PSUM on the right side: coexists with psC (left) during the
        # projection phase so pair-0 exp can start early.
        psS = L0.enter_context(
            tc.tile_pool(name="psS", bufs=2, space="PSUM", side="right"))

        eps_sb = consts.tile([P, 1], FP32)
        nc.vector.memset(eps_sb[:], EPS)
        ident = consts.tile([P, P], BF16)
        make_identity(nc, ident[:])

        def ln_group(tiles, mvtag, sink, t0, apply_eng=None):
            apply_eng = apply_eng or nc.vector
            g = len(tiles)
            mv = stat_pool.tile([P, g, 2], FP32, name=f"mv_{mvtag}", tag="mv")
            st6 = stat_pool.tile([P, 6], FP32, name=f"st6_{mvtag}", tag="st6")
            for i, xt in enumerate(tiles):
                nc.vector.bn_stats(st6[:], xt[:])
                nc.vector.bn_aggr(mv[:, i, :], st6[:])
            sd = stat_pool.tile([P, g], FP32, name=f"sd_{mvtag}", tag="sd")
            nc.scalar.activation(sd[:], mv[:, :, 1], Act.Sqrt, bias=eps_sb[:])
            rstd = stat_pool.tile([P, g], FP32, name=f"rstd_{mvtag}", tag="rstd")
            nc.vector.reciprocal_approx_fast(rstd[:], sd[:])
            mode, dest = sink
            for i, xt in enumerate(tiles):
                t = t0 + i
                zt = z_pool.tile([P, C], BF16, name=f"z_{mvtag}_{i}", tag="z")
                apply_eng.tensor_scalar(
                    zt[:], xt[:], mv[:, i, 0:1], rstd[:, i : i + 1],
                    Alu.subtract, Alu.mult,
                )
                if mode == "dram":
                    nc.sync.dma_start(dest[t], zt[:])
                else:
                    T_tiles, pspool, pstag = dest
                    for c in range(CT):
                        pt = pspool.tile([P, P], BF16, name=f"pt{c}", tag=pstag)
                        nc.tensor.transpose(
                            pt[:], zt[:, c * P : (c + 1) * P], ident[:]
                        )
                        nc.vector.tensor_copy(
                            T_tiles[c][:, t * P : (t + 1) * P], pt[:]
                        )

        def transpose_in(zdram, T_tiles, ntok):
            for c in range(CT):
                nc.sync.dma_start(
                    T_tiles[c][:, 0:ntok],
                    zdram[:, c * P : (c + 1) * P],
                    transpose=True,
                )

        x_own = [xo_pool.tile([P, C], FP32, name=f"xown{t}", tag="xown")
                 for t in range(NTO)]

        es_att = ExitStack()  # epool/rpool: right side, closed before MLP
        L2 = ExitStack()      # left-side projection-phase allocations
        try:
            zxT_p = L2.enter_context(tc.tile_pool(name="zxTp", bufs=CT))
            zxT = [zxT_p.tile([P, N], BF16, name=f"zxT{i}", tag="zxT")
                   for i in range(CT)]
            wB = L2.enter_context(tc.tile_pool(name="wB", bufs=1))
            psC = L2.enter_context(tc.tile_pool(name="psC", bufs=2, space="PSUM"))
            L2a = L2.enter_context(ExitStack())
            zeT_p = L2a.enter_context(tc.tile_pool(name="zeTp", bufs=CT))
            zeT = [zeT_p.tile([P, N], BF16, name=f"zeT{i}", tag="zeT")
                   for i in range(CT)]
            wsp = L2a.enter_context(tc.tile_pool(name="wsp", bufs=1))

            ze_dram = dram_p.tile([N, C], BF16, name="ze_dram")
            ze_dram_r = ze_dram[:].rearrange("(t p) c -> t p c", p=P)

            epool = es_att.enter_context(
                tc.tile_pool(name="epool", bufs=23, side="right"))
            E0 = {}

            def transpose_in_half(zdram, T_tiles, half):
                for c in range(CT):
                    nc.sync.dma_start(
                        T_tiles[c][:, half * TOK_OWN : (half + 1) * TOK_OWN],
                        zdram[half * TOK_OWN : (half + 1) * TOK_OWN,
                              c * P : (c + 1) * P],
                        transpose=True,
                    )

            # ---------- projections + early pair-0 scores ----------
            def q_proj(m):
                pq = psC.tile([P, TOK_OWN], FP32, name=f"pq{m}", tag="pc")
                for k in range(CT):
                    lw = wqkv_sb[:, k, m * P : (m + 1) * P]
                    for ch in range(NCH_OWN):
                        nc.tensor.matmul(
                            pq[:, ch * 512 : (ch + 1) * 512], lw,
                            zxT[k][:, ch * 512 : (ch + 1) * 512],
                            start=(k == 0), stop=(k == CT - 1),
                        )
                nc.vector.tensor_copy(qT[m][:], pq[:])

            def kk_half(m, half):
                # chunks (2*half, 2*half+1) of kk^T row-tile m
                pc = psC.tile([P, TOK_OWN], FP32, name=f"pc{m}_{half}", tag="pc")
                for k in range(CT):
                    lw = wqkv_sb[:, k, C + m * P : C + (m + 1) * P]
                    for i in range(2):
                        ch = 2 * half + i
                        nc.tensor.matmul(
                            pc[:, i * 512 : (i + 1) * 512], lw,
                            zxT[k][:, ch * 512 : (ch + 1) * 512],
                            start=(k == 0), stop=False,
                        )
                for k in range(CT):
                    lw = ws_sb[:, k, m * P : (m + 1) * P]
                    for i in range(2):
                        ch = 2 * half + i
                        nc.tensor.matmul(
                            pc[:, i * 512 : (i + 1) * 512], lw,
                            zeT[k][:, ch * 512 : (ch + 1) * 512],
                            start=False, stop=(k == CT - 1),
                        )
                nc.vector.tensor_copy(
                    kkT[m][:, half * TOK_OWN : (half + 1) * TOK_OWN], pc[:])

            def scores_exp(pr, t):
                scA = psS.tile([P, TOK_OWN], FP32, name="scA", tag="sc")
                scB = psS.tile([P, TOK_OWN], FP32, name="scB", tag="sc")
                for ch in range(NCH_OWN):
                    nc.tensor.matmul(
                        scA[:, ch * 512 : (ch + 1) * 512],
                        kkT[pr][0:D, t * P : (t + 1) * P],
                        qT[pr][0:D, ch * 512 : (ch + 1) * 512],
                        start=True, stop=True,
                    )
                    nc.tensor.matmul(
                        scB[:, ch * 512 : (ch + 1) * 512],
                        kkT[pr][D : 2 * D, t * P : (t + 1) * P],
                        qT[pr][D : 2 * D, ch * 512 : (ch + 1) * 512],
                        start=True, stop=True,
                    )
                EtA = epool.tile([P, TOK_OWN], BF16, name="EtA", tag="E")
                nc.scalar.activation(EtA[:], scA[:], Act.Exp)
                EtB = epool.tile([P, TOK_OWN], BF16, name="EtB", tag="E")
                nc.scalar.activation(EtB[:], scB[:], Act.Exp)
                return EtA, EtB

            def v_pair(tp):
                # token tiles 2*tp, 2*tp+1
                pv = psC.tile([P, TOK_OWN], FP32, name=f"pv{tp}", tag="pc")
                for k in range(CT):
                    for i in range(2):
                        t = 2 * tp + i
                        nc.tensor.matmul(
                            pv[:, i * 512 : (i + 1) * 512],
                            zxT[k][:, t * P : (t + 1) * P],
                            wqkv_sb[:, k, 2 * C : 3 * C],
                            start=(k == 0), stop=(k == CT - 1),
                        )
                for i in range(2):
                    t = 2 * tp + i
                    nc.vector.memset(v_aug[t][:, H * (D + 1) : VPAD], 0.0)
                    va = v_aug[t][:, 0 : H * (D + 1)].rearrange(
                        "p (h x) -> p h x", x=D + 1)
                    nc.vector.tensor_copy(
                        va[:, :, 0:D],
                        pv[:, i * 512 : (i + 1) * 512].rearrange(
                            "p (h d) -> p h d", d=D),
                    )
                    nc.vector.memset(va[:, :, D : D + 1], 1.0)

            def gate_proj(m):
                pg = psC.tile([P, TOK_OWN], FP32, name=f"pg{m}", tag="pc")
                for k in range(CT):
                    lw = wgate_sb[:, k, m * P : (m + 1) * P]
                    for ch in range(NCH_OWN):
                        nc.tensor.matmul(
                            pg[:, ch * 512 : (ch + 1) * 512], lw,
                            zxT[k][:, ch * 512 : (ch + 1) * 512],
                            start=(k == 0), stop=(k == CT - 1),
                        )
                nc.vector.tensor_copy(gTh[:, 2 * m, :], pg[0:D, :])
                nc.vector.tensor_copy(gTh[:, 2 * m + 1, :], pg[D : 2 * D, :])

            with (
                tc.tile_pool(name="xrp", bufs=5) as xr_pool,
                tc.tile_pool(name="ep", bufs=5) as e_pool,
            ):
                # interleave x/e group loads + LN so zeT half-0 (and with it
                # kk half-0 -> pair-0 scores) is ready as early as possible
                x_all = list(x_own)
                e_groups = []
                for g in range(4):
                    for t in range(4 * g, 4 * g + 4):
                        if t < NTO:
                            xt = x_all[t]
                        else:
                            xt = xr_pool.tile([P, C], FP32, name=f"xr{t}", tag="xr")
                            x_all.append(xt)
                        nc.sync.dma_start(xt[:], xb_r[t])
                    eg = []
                    for t in range(4 * g, 4 * g + 4):
                        et = e_pool.tile([P, C], FP32, name=f"e{t}", tag="e")
                        nc.sync.dma_start(et[:], eb_r[t])
                        eg.append(et)
                    e_groups.append(eg)
                    if g == 0:
                        wqkv_sb = wB.tile([P, CT, 3 * C], BF16)
                        nc.sync.dma_start(
                            wqkv_sb[:], wqkv_d.rearrange("(k p) n -> p k n", p=P))
                        ws_sb = wsp.tile([P, CT, C], BF16)
                        nc.sync.dma_start(
                            ws_sb[:], ws_d.rearrange("(k p) n -> p k n", p=P))
                    if g == 1:
                        wgate_sb = wB.tile([P, CT, C], BF16)
                        nc.sync.dma_start(
                            wgate_sb[:], wgate_d.rearrange("(k p) n -> p k n", p=P))

                for g in range(2):
                    ln_group(x_all[4 * g : 4 * g + 4], f"x{g}",
                             ("pe", (zxT, psC, "pc")), 4 * g)
                    ln_group(e_groups[g], f"e{g}", ("dram", ze_dram_r), 4 * g)
                transpose_in_half(ze_dram[:], zeT, 0)

                # pair-0 scores need only q0 + kk(0, half0): start the exp
                # stream now, then finish the LN groups underneath it
                q_proj(0)
                kk_half(0, 0)
                E0[0] = scores_exp(0, 0)
                E0[1] = scores_exp(0, 1)
                kk_half(1, 0)
                E0[2] = scores_exp(0, 2)
                kk_half(2, 0)
                E0[3] = scores_exp(0, 3)
                kk_half(3, 0)
                E0[4] = scores_exp(0, 4)
                v_pair(0)
                ln_group(x_all[8:12], "x2", ("pe", (zxT, psC, "pc")), 8)
                E0[5] = scores_exp(0, 5)
                v_pair(1)
                ln_group(e_groups[2], "e2", ("dram", ze_dram_r), 8)
                E0[6] = scores_exp(0, 6)
                v_pair(2)
                q_proj(1)
                ln_group(x_all[12:16], "x3", ("pe", (zxT, psC, "pc")), 12)
                E0[7] = scores_exp(0, 7)
                v_pair(3)
                q_proj(2)
                ln_group(e_groups[3], "e3", ("dram", ze_dram_r), 12)
                transpose_in_half(ze_dram[:], zeT, 1)

            N_EARLY = 10
            kk_half(0, 1)
            E0[8] = scores_exp(0, 8)
            E0[9] = scores_exp(0, 9)
            kk_half(1, 1)
            kk_half(2, 1)
            kk_half(3, 1)
            q_proj(3)
            L2a.close()  # zeT, ws freed
            for tp in range(4, NT // 2):
                v_pair(tp)
            for m in range(CT):
                gate_proj(m)
        finally:
            L2.close()  # zxT, wqkv, wgate, psC freed

        rpool = es_att.enter_context(
            tc.tile_pool(name="rpool", bufs=2, side="right"))
        xm_pool = L0.enter_context(tc.tile_pool(name="xm", bufs=NTO))

        # ---------- attention (+ MLP reusing the same PSUM pools) ----------
        with tc.tile_pool(name="psO", bufs=2, space="PSUM") as psO:

            def normalize(ps_o, h):
                dn = rpool.tile([1, TOK_OWN], FP32, name="dn", tag="dn")
                nc.vector.tensor_copy(dn[:], ps_o[D : D + 1, :])
                t1 = rpool.tile([D, TOK_OWN], FP32, name="t1", tag="t1")
                nc.vector.tensor_mul(t1[:], ps_o[0:D, :], gTh[:, h, :])
                dnb = rpool.tile([D, TOK_OWN], FP32, name="dnb", tag="dnb")
                nc.gpsimd.partition_broadcast(dnb[:], dn[:])
                rdb = rpool.tile([D, TOK_OWN], FP32, name="rdb", tag="rdb")
                nc.vector.reciprocal_approx_fast(rdb[:], dnb[:])
                kt, po = h // 2, (h % 2) * D
                nc.vector.tensor_mul(ogT[kt][po : po + D, :], t1[:], rdb[:])

            # Flat lag-pipelined loop: scores/exp run LAG tiles ahead of
            # attn@v so the ACT exp stream (the pacer) never stalls on the
            # psO accumulator handoff between head pairs.
            from collections import deque

            seq = [(pr, t) for pr in range(CT) for t in range(NT)]
            pending = deque(((0, t), E0.pop(t)) for t in range(N_EARLY))
            ps_now = {}

            def emit_attnv(entry, Ets):
                pr, t = entry
                hA, hB = 2 * pr, 2 * pr + 1
                if t == 0:
                    ps_now[0] = psO.tile([P, TOK_OWN], FP32,
                                         name=f"psoA{pr}", tag="po")
                    ps_now[1] = psO.tile([P, TOK_OWN], FP32,
                                         name=f"psoB{pr}", tag="po")
                EtA, EtB = Ets
                for ch in range(NCH_OWN):
                    nc.tensor.matmul(
                        ps_now[0][:, ch * 512 : (ch + 1) * 512],
                        v_aug[t][:, hA * (D + 1) : hA * (D + 1) + P],
                        EtA[:, ch * 512 : (ch + 1) * 512],
                        start=(t == 0), stop=(t == NT - 1),
                    )
                    nc.tensor.matmul(
                        ps_now[1][:, ch * 512 : (ch + 1) * 512],
                        v_aug[t][:, hB * (D + 1) : hB * (D + 1) + P],
                        EtB[:, ch * 512 : (ch + 1) * 512],
                        start=(t == 0), stop=(t == NT - 1),
                    )
                if t == NT - 1:
                    normalize(ps_now[0], hA)
                    normalize(ps_now[1], hB)

            for entry in seq[N_EARLY:]:
                pending.append((entry, scores_exp(*entry)))
                e2, Ets = pending.popleft()
                emit_attnv(e2, Ets)
            while pending:
                e2, Ets = pending.popleft()
                emit_attnv(e2, Ets)

            es_att.close()  # epool, rpool freed before the MLP needs SBUF

            # ---------- proj + residual + LN3 + MLP ----------
            with (
                tc.tile_pool(name="wE", bufs=1) as wE,
                tc.tile_pool(name="z3Tp", bufs=CT) as z3T_pool,
                tc.tile_pool(name="hTp", bufs=DFT) as hT_pool,
                tc.tile_pool(name="opool", bufs=4) as opool,
            ):
                wproj_sb = wE.tile([P, CT, C], BF16)
                nc.sync.dma_start(wproj_sb[:],
                                  wproj_d.rearrange("(k p) n -> p k n", p=P))
                wfc1_sb = wE.tile([P, CT, DFF], BF16)
                nc.sync.dma_start(wfc1_sb[:],
                                  wfc1_d.rearrange("(k p) n -> p k n", p=P))
                wfc2_sb = wE.tile([P, DFT, C], BF16)
                nc.sync.dma_start(wfc2_sb[:],
                                  wfc2_d.rearrange("(k p) n -> p k n", p=P))

                xm = []
                for tp in range(NTO // 2):
                    pp = psO.tile([P, TOK_OWN], FP32, name=f"pp{tp}", tag="po")
                    for i in range(2):
                        t = 2 * tp + i
                        for k in range(CT):
                            nc.tensor.matmul(
                                pp[:, i * 512 : (i + 1) * 512],
                                ogT[k][:, t * P : (t + 1) * P], wproj_sb[:, k, :],
                                start=(k == 0), stop=(k == CT - 1),
                            )
                    for i in range(2):
                        t = 2 * tp + i
                        xmt = xm_pool.tile([P, C], FP32, name=f"xm{t}", tag="xm")
                        nc.vector.tensor_add(
                            xmt[:], x_own[t][:], pp[:, i * 512 : (i + 1) * 512])
                        xm.append(xmt)

                z3T = [z3T_pool.tile([P, TOK_OWN], BF16, name=f"z3T{i}", tag="z3Tt")
                       for i in range(CT)]
                for g0 in range(0, NTO, 4):
                    ln_group(xm[g0 : g0 + 4], f"x3{g0}", ("pe", (z3T, psS, "sc")), g0)

                # fc1 + gelu + fc2 in two half-width token-chunk passes:
                # chunk ch of fc1 needs only LN3-group-ch's z3T columns, and
                # the fc2 accumulation for tokens 4ch..4ch+3 needs only the
                # gelu output of chunk ch -- so each pass starts as soon as
                # its LN3 group lands and fc2 fully hides under fc1.
                hT = [hT_pool.tile([P, TOK_OWN], BF16, name=f"hT{i}", tag="hTt")
                      for i in range(DFT)]
                for ch in range(NCH_OWN):
                    sl = slice(ch * 512, (ch + 1) * 512)
                    pf2w = [psO.tile([P, TOK_OWN], FP32, name=f"pf2w{ch}{i}",
                                     tag="po") for i in range(2)]
                    for m in range(DFT):
                        pf = psS.tile([P, 512], FP32, name=f"pf{ch}", tag="sc")
                        for k in range(CT):
                            nc.tensor.matmul(
                                pf[:], wfc1_sb[:, k, m * P : (m + 1) * P],
                                z3T[k][:, sl],
                                start=(k == 0), stop=(k == CT - 1),
                            )
                        nc.scalar.activation(hT[m][:, sl], pf[:], Act.Gelu)
                        for i in range(4):
                            t = 4 * ch + i
                            nc.tensor.matmul(
                                pf2w[i // 2][:, (i % 2) * 512 : (i % 2 + 1) * 512],
                                hT[m][:, t * P : (t + 1) * P], wfc2_sb[:, m, :],
                                start=(m == 0), stop=(m == DFT - 1),
                            )
                    for i in range(4):
                        t = 4 * ch + i
                        ot = opool.tile([P, C], FP32, name="ot", tag="ot")
                        nc.vector.tensor_add(
                            ot[:], xm[t][:],
                            pf2w[i // 2][:, (i % 2) * 512 : (i % 2 + 1) * 512])
                        nc.sync.dma_start(out_r[t], ot[:])


def _preprocess(inputs):
    """Fold LN affine + attention scale into weights (host-side, weight-only)."""
    f32 = np.float32
    ln1_w, ln1_b = f32(inputs["ln1_w"]), f32(inputs["ln1_b"])
    ln2_w, ln2_b = f32(inputs["ln2_w"]), f32(inputs["ln2_b"])
    ln3_w, ln3_b = f32(inputs["ln3_w"]), f32(inputs["ln3_b"])
    w_qkv = f32(inputs["w_qkv"]).copy()
    w_s = f32(inputs["w_s"])
    w_gate = f32(inputs["w_gate"])
    w_proj = f32(inputs["w_proj"])
    w_fc1 = f32(inputs["w_fc1"])
    w_fc2 = f32(inputs["w_fc2"])

    scale = D ** -0.5
    wqkv_eff = ln1_w[:, None] * w_qkv
    wqkv_eff[:, 0:C] *= scale
    b_qkv = ln1_b @ w_qkv
    b_qkv[0:C] *= scale
    ws_eff = ln2_w[:, None] * w_s
    b_s = ln2_b @ w_s
    wgate_eff = ln1_w[:, None] * w_gate
    b_gate = ln1_b @ w_gate
    wfc1_eff = ln3_w[:, None] * w_fc1
    b_fc1 = ln3_b @ w_fc1 + f32(inputs["b_fc1"])

    for name, bias in [
        ("b_qkv", b_qkv), ("b_s", b_s), ("b_gate", b_gate), ("b_fc1", b_fc1),
        ("b_proj", f32(inputs["b_proj"])), ("b_fc2", f32(inputs["b_fc2"])),
    ]:
        assert np.all(bias == 0.0), f"nonzero bias {name} unsupported by this kernel"

    bf16 = ml_dtypes.bfloat16
    return {
        "wqkv": np.ascontiguousarray(wqkv_eff, dtype=bf16),
        "ws": np.ascontiguousarray(ws_eff, dtype=bf16),
        "wgate": np.ascontiguousarray(wgate_eff, dtype=bf16),
        "wproj": np.ascontiguousarray(w_proj, dtype=bf16),
        "wfc1": np.ascontiguousarray(wfc1_eff, dtype=bf16),
        "wfc2": np.ascontiguousarray(w_fc2, dtype=bf16),
    }


def kernel(**inputs):
    from concourse import bass_utils

    if "nc" not in _CACHE:
        _CACHE["nc"] = _build_nc()
    nc = _CACHE["nc"]

    w = _preprocess(inputs)
    x = np.asarray(inputs["x"], dtype=np.float32)
    e = np.asarray(inputs["e"], dtype=np.float32)

    in_maps = []
    for c in range(N_CORES):
        b, half = c // 2, c % 2
        if half == 0:
            xb, eb = x[b], e[b]
        else:
            xb = np.concatenate([x[b, TOK_OWN:], x[b, :TOK_OWN]], axis=0)
            eb = np.concatenate([e[b, TOK_OWN:], e[b, :TOK_OWN]], axis=0)
        in_maps.append({
            "xb": np.ascontiguousarray(xb),
            "eb": np.ascontiguousarray(eb),
            **w,
        })

    res = bass_utils.run_bass_kernel_spmd(
        nc, in_maps, core_ids=list(range(N_CORES)),
        trace=_CACHE.get("trace", False),
    )
    _CACHE["last_result"] = res

    out = np.empty((B, N, C), dtype=np.float32)
    for c in range(N_CORES):
        b, half = c // 2, c % 2
        out[b, half * TOK_OWN : (half + 1) * TOK_OWN] = res.results[c]["out"]
    return out

